# revision 25
# baseline (speedup 1.0000x reference)
"""Trainium2 Bass kernel for nn_MetaFunUpdaterLocal (gnn_message_passing).

Math (per meta-batch b, per outer-tile k):
    h    = concat([x[b], y[b], r_c[b,k]], -1)           [C, 160]
    U    = MLP(h)  (160->128 relu ->128 relu ->64)      [C, 64]
    next_r_c[b,k] = r_c[b,k] - 0.1 * c_att[b] @ U
    next_r_q[b,k] = r_q[b,k] - 0.1 * q_att[b] @ U

Design (vs the 332 us fp32r baseline):
  * Everything on the wire is bf16: r tiles, constants, outputs. Halves
    HBM traffic (memory-regime target) and descriptor count.
  * All matmuls bf16 with M=128 stationaries -> FWL (fast weight load):
    ~96 ns per 512-row matmul vs ~190 ns for the fp32r baseline.
  * r tile layout [128, 512] per 2-pair group: partitions 0:64 = pair A
    features, 64:128 = pair B; free 0:256 = rc cols, 256:512 = rq cols.
    Layer 1 reads this WITHOUT any SBUF->SBUF remap: per pair, one K=128
    matmul with a zero-padded stationary ([W1d;0] for A, [0;W1d] for B)
    contracts the full partition dim; the other pair's rows hit zeros.
  * P[b] = [x|y]@W1[:96] + b1 (k-independent layer-1 part) is precomputed
    on host and injected into PSUM with a bf16 identity matmul.
  * b3 folded on host into the shipped r tiles (rank-1 correction
    -0.1 * att_rowsum x b3). b2 rides the s2 relu as per-partition bias.
  * Delta matmuls ordered so d0,d2 share the u0 stationary and d1,d3
    share u1 -> 2 stationary loads instead of 4.
  * ONE [128,1024] PSUM tile (2 banks) per iteration serves BOTH groups'
    z1 -> z2 -> ups -> dp: each reuse is a write-after-read already
    ordered by the dataflow (z2 needs s1, L3 needs s2, deltas need u),
    so 4 bufs = 8 banks = 4 iterations (8 groups) in flight. PSUM banks
    are the pipeline-depth limiter.
  * The relus and the final update-add are ONE [128,1024] instruction per
    iteration (covering both groups): the Act engine is the saturated
    resource (~80 us busy) and per-instruction overhead is ~185 ns, so
    halving the instruction count buys ~12 us. (Splitting a relu between
    Act and DVE instead REGRESSES: two engines reading the same PSUM
    tile couple into the next matmul's WAR chain.)
  * Loads issue on the SP (sync) HWDGE ring, stores on GpSimd SWDGE --
    store waits no longer head-of-line-block the next loads on the
    in-order SP stream (this blocking was why the baseline ran at 332 us
    with no engine above 46% busy).

Sharding: 8 cores, core c handles b = c//2 and a 128-pair slice of the
outer C axis (B x outer-C data parallel per the sharding hint).
"""

import numpy as np

B, C, Q, XD, YD, E, H = 4, 256, 256, 64, 32, 64, 128
NCORES = 8
NG_CORE = 64   # 2-pair groups per core
NIT = 32       # iterations; each handles 2 groups (one 256 KiB IO tile)

_NC_CACHE = {}

# cbig bf16 constant layout (cols):
#   [0:512]     pt2   (P[b].T duplicated for both pairs)
#   [512:1024]  ac    (-0.1 * c_attT, j-chunked)
#   [1024:1536] aq
#   [1536:1664] i128
#   [1664:1792] w1dA  (rows 0:64  = W1[96:160], rows 64:128 = 0)
#   [1792:1920] w1dB  (rows 0:64 = 0, rows 64:128 = W1[96:160])
#   [1920:2048] w2
#   [2048:2112] w3
#   [2112:2114] b2 as 2 bf16 cols bitcast-> fp32 [128,1]
CB = 2114


def _build_nc(nit=NIT):
    import concourse.bass as bass
    import concourse.bacc as bacc
    import concourse.mybir as mybir
    import concourse.tile as tile
    from concourse.bass import _add_dep_helper

    F32 = mybir.dt.float32
    BF16 = mybir.dt.bfloat16
    ADD = mybir.AluOpType.add
    RELU = mybir.ActivationFunctionType.Relu

    nc = bacc.Bacc("TRN2", target_bir_lowering=False, debug=False,
                   num_devices=NCORES)

    rt_d = nc.dram_tensor("rt", [nit, 128, 1024], BF16, kind="ExternalInput")
    cb_d = nc.dram_tensor("cbig", [128, CB], BF16, kind="ExternalInput")
    out_d = nc.dram_tensor("out", [nit, 128, 1024], BF16, kind="ExternalOutput")

    with tile.TileContext(nc) as tc:
        with (
            tc.tile_pool(name="const", bufs=1) as cp,
            tc.tile_pool(name="rt", bufs=6) as rtp,
            tc.tile_pool(name="s1", bufs=4) as s1p,
            tc.tile_pool(name="s2", bufs=4) as s2p,
            tc.tile_pool(name="u", bufs=4) as up,
            tc.tile_pool(name="o", bufs=4) as op,
            tc.tile_pool(name="pzu", bufs=4, space="PSUM") as pzu,
        ):
            cbig = cp.tile([128, CB], BF16)
            # const load on the Act HWDGE ring so it overlaps the first rt
            # load on the SP ring
            nc.scalar.dma_start(cbig[:], cb_d[:, :])
            pt2 = cbig[:, 0:512]
            ac = cbig[:, 512:1024]
            aq = cbig[:, 1024:1536]
            i128 = cbig[:, 1536:1664]
            w1dA = cbig[:, 1664:1792]
            w1dB = cbig[:, 1792:1920]
            w2 = cbig[:, 1920:2048]
            w3 = cbig[:, 2048:2112]
            b2ap = cbig[:, 2112:2114].bitcast(F32)

            def chain(mms):
                for a, b_ in zip(mms[1:], mms):
                    _add_dep_helper(a.ins, b_.ins, sync=False, reason="psum order")

            for it in range(nit):
                rt2 = rtp.tile([128, 1024], BF16)
                nc.sync.dma_start(rt2[:], rt_d[it, :, :])
                o4 = op.tile([128, 1024], BF16)

                # one 2-bank PSUM tile serves z1 -> z2 -> ups -> dp for BOTH
                # halves; every reuse is write-after-read ordered by dataflow
                zz = pzu.tile([128, 1024], F32)

                # ---- layer 1 (both halves): z = P inject + W1d.T @ rcT
                mms = []
                for h in range(2):
                    o = h * 512
                    m0 = nc.tensor.matmul(zz[:, o:o + 512], i128[:], pt2[:],
                                          start=True, stop=False)
                    m1a = nc.tensor.matmul(zz[:, o:o + 256], w1dA[:],
                                           rt2[:, o:o + 256],
                                           start=False, stop=False)
                    m1b = nc.tensor.matmul(zz[:, o + 256:o + 512], w1dB[:],
                                           rt2[:, o:o + 256],
                                           start=False, stop=True)
                    mms += [m0, m1a, m1b]
                chain(mms)
                s1 = s1p.tile([128, 1024], BF16)
                nc.scalar.activation(s1[:], zz[:], RELU)

                # ---- layer 2 (both halves) reuses zz
                za = nc.tensor.matmul(zz[:, 0:512], w2[:], s1[:, 0:512],
                                      start=True, stop=True)
                zb = nc.tensor.matmul(zz[:, 512:1024], w2[:], s1[:, 512:1024],
                                      start=True, stop=True)
                chain([za, zb])
                s2 = s2p.tile([128, 1024], BF16)
                nc.scalar.activation(s2[:], zz[:], RELU, bias=b2ap)

                # ---- layer 3 into zz[:, h*512 : h*512+256]:
                # cols per half: [A-ch0 | B-ch0 | A-ch1 | B-ch1]
                umms = []
                for h in range(2):
                    o = h * 512
                    for ch in range(2):
                        for p in range(2):
                            mm = nc.tensor.matmul(
                                zz[:, o + ch * 128 + p * 64:
                                   o + ch * 128 + p * 64 + 64],
                                s2[:, o + p * 256 + ch * 128:
                                   o + p * 256 + (ch + 1) * 128],
                                w3[:],
                                start=(ch == 0 and p == 0),
                                stop=(ch == 1 and p == 1))
                            umms.append(mm)
                chain(umms)
                u = up.tile([128, 512], BF16)
                nc.vector.tensor_copy(u[:, 0:256], zz[:, 0:256])
                nc.vector.tensor_copy(u[:, 256:512], zz[:, 512:768])

                # ---- deltas overwrite zz (they depend on the u casts);
                # per half: d0,d2 share stationary u0; d1,d3 share u1
                dms = []
                for h in range(2):
                    o = h * 512
                    uo = h * 256
                    d0 = nc.tensor.matmul(zz[:, o:o + 256], u[:, uo:uo + 128],
                                          ac[:, 0:256],
                                          start=True, stop=False)
                    d2 = nc.tensor.matmul(zz[:, o + 256:o + 512],
                                          u[:, uo:uo + 128], aq[:, 0:256],
                                          start=False, stop=False)
                    d1 = nc.tensor.matmul(zz[:, o:o + 256],
                                          u[:, uo + 128:uo + 256],
                                          ac[:, 256:512],
                                          start=False, stop=False)
                    d3 = nc.tensor.matmul(zz[:, o + 256:o + 512],
                                          u[:, uo + 128:uo + 256],
                                          aq[:, 256:512],
                                          start=False, stop=True)
                    dms += [d0, d2, d1, d3]
                chain(dms)

                # ---- update (both halves in one op) + store
                nc.vector.tensor_tensor(o4[:], rt2[:], zz[:], op=ADD)
                nc.gpsimd.dma_start(out_d[it, :, :], o4[:])

    nc.finalize()
    return nc


def _get_nc(nit=NIT):
    if nit not in _NC_CACHE:
        _NC_CACHE[nit] = _build_nc(nit)
    return _NC_CACHE[nit]


def _to_bf16(x):
    import ml_dtypes
    return np.asarray(x, dtype=ml_dtypes.bfloat16)


def _host_prep(x, y, r_c, r_q, c_att_map, q_att_map, W1, b1, W2, b2, W3, b3):
    """Build per-core input maps. Returns in_maps."""
    import ml_dtypes
    bf = ml_dtypes.bfloat16
    f32 = np.float32
    x = np.asarray(x, f32); y = np.asarray(y, f32)
    r_c = np.asarray(r_c, f32)
    r_q = np.asarray(r_q, f32)
    c_att = np.asarray(c_att_map, f32); q_att = np.asarray(q_att_map, f32)
    W1 = np.asarray(W1, f32); b1 = np.asarray(b1, f32)
    W2 = np.asarray(W2, f32); b2 = np.asarray(b2, f32)
    W3 = np.asarray(W3, f32); b3 = np.asarray(b3, f32)

    # fold b3 into the shipped r tiles: next_r = (r - 0.1*rowsum x b3) - 0.1*att@U'
    if np.any(b3):
        r_c = r_c - 0.1 * c_att.sum(axis=2)[:, None, :, None] * b3
        r_q = r_q - 0.1 * q_att.sum(axis=2)[:, None, :, None] * b3

    # P[b] = [x|y] @ W1[:96] + b1  (k-independent part of layer 1), transposed
    xy = np.concatenate([x, y], axis=-1)                      # [B, C, 96]
    P = xy @ W1[:XD + YD] + b1                                # [B, C, H]
    PT = np.ascontiguousarray(P.transpose(0, 2, 1))           # [B, H, C]
    pt2 = _to_bf16(np.concatenate([PT, PT], axis=2))          # [B, 128, 512]

    # rt[b, g] = [[rcT(2g); rcT(2g+1)] | [rqT(2g); rqT(2g+1)]]  [128, 512]
    rc2 = np.ascontiguousarray(r_c.transpose(0, 1, 3, 2)).reshape(B, C // 2, 128, 256)
    rq2 = np.ascontiguousarray(r_q.transpose(0, 1, 3, 2)).reshape(B, C // 2, 128, 256)
    rt = _to_bf16(np.concatenate([rc2, rq2], axis=3))         # [B, G, 128, 512]

    # attention maps: transposed, chunked along j, pre-scaled by -0.1
    def att_chunks(a):  # [B, i, j] -> [B, 128, 512] = [-0.1*aT ch0 | ch1]
        at = (-0.1 * a.transpose(0, 2, 1)).astype(f32)        # [B, j, i]
        return _to_bf16(np.ascontiguousarray(
            at.reshape(B, 2, 128, 256).transpose(0, 2, 1, 3)).reshape(B, 128, 512))

    acs = att_chunks(c_att)
    aqs = att_chunks(q_att)

    i128 = np.eye(128, dtype=bf)
    w1dA = np.zeros((128, 128), dtype=bf)
    w1dA[:64] = _to_bf16(W1[XD + YD:])
    w1dB = np.zeros((128, 128), dtype=bf)
    w1dB[64:] = _to_bf16(W1[XD + YD:])
    b2_as_bf = np.ascontiguousarray(b2.astype(f32)).view(np.uint16).reshape(128, 2)

    in_maps = []
    for core in range(NCORES):
        b = core // 2
        g0 = (core % 2) * NG_CORE
        cbig = np.zeros((128, CB), dtype=bf)
        cbig[:, 0:512] = pt2[b]
        cbig[:, 512:1024] = acs[b]
        cbig[:, 1024:1536] = aqs[b]
        cbig[:, 1536:1664] = i128
        cbig[:, 1664:1792] = w1dA
        cbig[:, 1792:1920] = w1dB
        cbig[:, 1920:2048] = _to_bf16(W2)
        cbig[:, 2048:2112] = _to_bf16(W3)
        cbig[:, 2112:2114] = b2_as_bf.view(bf)
        # pack 2 consecutive groups side by side on the free dim
        rt_core = rt[b, g0:g0 + NG_CORE].reshape(NIT, 2, 128, 512)
        rt_core = np.ascontiguousarray(
            rt_core.transpose(0, 2, 1, 3)).reshape(NIT, 128, 1024)
        in_maps.append({
            "rt": rt_core,
            "cbig": cbig,
        })
    return in_maps


def _host_post(results):
    """results[core]["out"] [NIT, 128, 1024] bf16 -> (next_r_c, next_r_q)."""
    next_r_c = np.empty((B, C, C, E), np.float32)
    next_r_q = np.empty((B, C, C, E), np.float32)
    for core in range(NCORES):
        out = np.asarray(results[core]["out"]).astype(np.float32)
        out = out.reshape(NIT, 128, 2, 512).transpose(0, 2, 1, 3)
        out = out.reshape(NG_CORE, 128, 512)
        b = core // 2
        k0 = (core % 2) * 128
        rc = out[:, :, 0:256].reshape(NG_CORE, 2, 64, 256)
        rq = out[:, :, 256:512].reshape(NG_CORE, 2, 64, 256)
        next_r_c[b, k0:k0 + 128] = rc.transpose(0, 1, 3, 2).reshape(128, 256, 64)
        next_r_q[b, k0:k0 + 128] = rq.transpose(0, 1, 3, 2).reshape(128, 256, 64)
    return next_r_c, next_r_q


def kernel(x, y, r_c, r_q, c_att_map, q_att_map, W1, b1, W2, b2, W3, b3,
           _trace=False, _trace_kwargs=None):
    import time
    from concourse.bass_utils import run_bass_kernel_spmd

    t0 = time.time()
    nc = _get_nc()
    t1 = time.time()
    in_maps = _host_prep(x, y, r_c, r_q, c_att_map, q_att_map,
                         W1, b1, W2, b2, W3, b3)
    t2 = time.time()
    res = run_bass_kernel_spmd(
        nc, in_maps, list(range(NCORES)),
        trace=_trace, **(_trace_kwargs or {}))
    t3 = time.time()
    out = _host_post(res.results)
    t4 = time.time()
    kernel.last_result = res
    kernel.timings = {"build": t1 - t0, "prep": t2 - t1, "run": t3 - t2,
                      "post": t4 - t3}
    return out


# revision 26
# speedup vs baseline: 1.1020x; 1.1020x over previous
"""Trainium2 Bass kernel for nn_MetaFunUpdaterLocal (gnn_message_passing).

Math (per meta-batch b, per outer-tile k):
    h    = concat([x[b], y[b], r_c[b,k]], -1)           [C, 160]
    U    = MLP(h)  (160->128 relu ->128 relu ->64)      [C, 64]
    next_r_c[b,k] = r_c[b,k] - 0.1 * c_att[b] @ U
    next_r_q[b,k] = r_q[b,k] - 0.1 * q_att[b] @ U

v2.1 design (vs the fp32 baseline):
  * Everything on the wire is bf16: r tiles, constants, outputs. Halves
    HBM traffic (memory-regime target) and descriptor count.
  * All matmuls bf16 with M=128 stationaries -> FWL (fast weight load)
    removes most LDWEIGHTS overhead vs the fp32r baseline.
  * r tile layout [128, 512] (same as baseline): partitions 0:64 = pair A
    features, 64:128 = pair B; free 0:256 = rc cols, 256:512 = rq cols.
    Layer 1 reads this WITHOUT the baseline's SBUF->SBUF remap: per pair,
    one K=128 matmul with a zero-padded stationary ([W1d;0] for pair A,
    [0;W1d] for pair B) contracts the full partition dim; the other
    pair's rows multiply zeros.
  * P[b] = [x|y]@W1[:96] + b1 (k-independent layer-1 part) is precomputed
    on host and injected into PSUM with a bf16 identity matmul.
  * b3 folded on host into the shipped r tiles (rank-1 correction
    -0.1 * att_rowsum x b3). b2 rides the s2 relu as per-partition bias.
  * Delta matmuls ordered so d0,d2 share the u0 stationary and d1,d3
    share u1 -> 2 stationary loads instead of 4.
  * IO tiles span 2 groups (4 pairs, 256 KiB): loads issue on the SP
    (sync) HWDGE ring, stores on GpSimd SWDGE -- store waits no longer
    head-of-line-block the next loads on the in-order SP stream.
  * Engine balance: Act = both relus, DVE = u copy + final update add.

Sharding: 8 cores, core c handles b = c//2 and a 128-pair slice of the
outer C axis (B x outer-C data parallel per the sharding hint).
"""

import numpy as np

B, C, Q, XD, YD, E, H = 4, 256, 256, 64, 32, 64, 128
NCORES = 8
NG_CORE = 64   # 2-pair groups per core
NIT = 32       # iterations; each handles 2 groups (one 256 KiB IO tile)

_NC_CACHE = {}

# cbig bf16 constant layout (cols):
#   [0:512]     pt2   (P[b].T duplicated for both pairs)
#   [512:1024]  ac    (-0.1 * c_attT, j-chunked)
#   [1024:1536] aq
#   [1536:1664] i128
#   [1664:1792] w1dA  (rows 0:64  = W1[96:160], rows 64:128 = 0)
#   [1792:1920] w1dB  (rows 0:64 = 0, rows 64:128 = W1[96:160])
#   [1920:2048] w2
#   [2048:2112] w3
#   [2112:2114] b2 as 2 bf16 cols bitcast-> fp32 [128,1]
CB = 2114


def _build_nc(nit=NIT):
    import concourse.bass as bass
    import concourse.bacc as bacc
    import concourse.mybir as mybir
    import concourse.tile as tile
    from concourse.bass import _add_dep_helper

    F32 = mybir.dt.float32
    BF16 = mybir.dt.bfloat16
    FP8 = mybir.dt.float8e4
    ADD = mybir.AluOpType.add
    RELU = mybir.ActivationFunctionType.Relu
    DR = mybir.MatmulPerfMode.DoubleRow

    nc = bacc.Bacc("TRN2", target_bir_lowering=False, debug=False,
                   num_devices=NCORES)

    rt_d = nc.dram_tensor("rt", [nit, 128, 1024], BF16, kind="ExternalInput")
    cb_d = nc.dram_tensor("cbig", [128, CB], BF16, kind="ExternalInput")
    out_d = nc.dram_tensor("out", [nit, 128, 1024], BF16, kind="ExternalOutput")

    with tile.TileContext(nc) as tc:
        with (
            tc.tile_pool(name="const", bufs=1) as cp,
            tc.tile_pool(name="rt", bufs=6) as rtp,
            tc.tile_pool(name="s1", bufs=6) as s1p,
            tc.tile_pool(name="s2", bufs=6) as s2p,
            tc.tile_pool(name="u", bufs=6) as up,
            tc.tile_pool(name="o", bufs=4) as op,
            tc.tile_pool(name="pzu", bufs=8, space="PSUM") as pzu,
        ):
            cbig = cp.tile([128, CB], BF16)
            nc.sync.dma_start(cbig[:], cb_d[:, :])
            pt2 = cbig[:, 0:512]
            ac = cbig[:, 512:1024]
            aq = cbig[:, 1024:1536]
            i128 = cbig[:, 1536:1664]
            w1dA = cbig[:, 1664:1792]
            w1dB = cbig[:, 1792:1920]
            w2 = cbig[:, 1920:2048]
            w3 = cbig[:, 2048:2112]
            b2ap = cbig[:, 2112:2114].bitcast(F32)

            def chain(mms):
                for a, b_ in zip(mms[1:], mms):
                    _add_dep_helper(a.ins, b_.ins, sync=False, reason="psum order")

            for it in range(nit):
                rt2 = rtp.tile([128, 1024], BF16)
                nc.sync.dma_start(rt2[:], rt_d[it, :, :])
                o4 = op.tile([128, 1024], BF16)

                for half in range(2):
                    rt = rt2[:, half * 512: half * 512 + 512]
                    o2 = o4[:, half * 512: half * 512 + 512]

                    # ---- one PSUM tile serves z1 -> z2 -> ups -> dp: every
                    # reuse is write-after-read already ordered by dataflow,
                    # so 8 bufs = 8 halves in flight
                    z = pzu.tile([128, 512], F32)
                    m0 = nc.tensor.matmul(z[:], i128[:], pt2[:],
                                          start=True, stop=False)
                    m1a = nc.tensor.matmul(z[:, 0:256], w1dA[:],
                                           rt[:, 0:256], start=False, stop=False)
                    m1b = nc.tensor.matmul(z[:, 256:512], w1dB[:],
                                           rt[:, 0:256], start=False, stop=True)
                    chain([m0, m1a, m1b])
                    s1 = s1p.tile([128, 512], BF16)
                    nc.scalar.activation(s1[:], z[:], RELU)

                    # ---- layer 2 reuses z's PSUM bank (z2 already depends on
                    # relu1 through s1, so the WAR reuse costs nothing)
                    nc.tensor.matmul(z[:], w2[:], s1[:], start=True, stop=True)
                    s2 = s2p.tile([128, 512], BF16)
                    nc.scalar.activation(s2[:], z[:], RELU, bias=b2ap)

                    # ---- layer 3 into ud[:, 0:256]: [A-ch0 | B-ch0 | A-ch1 | B-ch1]
                    ud = z
                    umms = []
                    for ch in range(2):
                        for p in range(2):
                            mm = nc.tensor.matmul(
                                ud[:, ch * 128 + p * 64: ch * 128 + p * 64 + 64],
                                s2[:, p * 256 + ch * 128: p * 256 + (ch + 1) * 128],
                                w3[:],
                                start=(ch == 0 and p == 0),
                                stop=(ch == 1 and p == 1))
                            umms.append(mm)
                    chain(umms)
                    u = up.tile([128, 256], BF16)
                    nc.vector.tensor_copy(u[:], ud[:, 0:256])

                    # ---- deltas overwrite ud (they depend on the u cast);
                    # d0,d2 share stationary u0; d1,d3 share u1
                    d0 = nc.tensor.matmul(ud[:, 0:256], u[:, 0:128],
                                          ac[:, 0:256], start=True, stop=False)
                    d2 = nc.tensor.matmul(ud[:, 256:512], u[:, 0:128],
                                          aq[:, 0:256], start=False, stop=False)
                    d1 = nc.tensor.matmul(ud[:, 0:256], u[:, 128:256],
                                          ac[:, 256:512], start=False, stop=False)
                    d3 = nc.tensor.matmul(ud[:, 256:512], u[:, 128:256],
                                          aq[:, 256:512], start=False, stop=True)
                    chain([d0, d2, d1, d3])

                    # ---- update
                    nc.vector.tensor_tensor(o2[:], rt[:], ud[:], op=ADD)

                # store both halves with one SWDGE dma (keeps the SP ring
                # free of store waits)
                nc.gpsimd.dma_start(out_d[it, :, :], o4[:])

    nc.finalize()
    return nc


def _get_nc(nit=NIT):
    if nit not in _NC_CACHE:
        _NC_CACHE[nit] = _build_nc(nit)
    return _NC_CACHE[nit]


def _to_bf16(x):
    import ml_dtypes
    return np.asarray(x, dtype=ml_dtypes.bfloat16)


def _host_prep(x, y, r_c, r_q, c_att_map, q_att_map, W1, b1, W2, b2, W3, b3):
    """Build per-core input maps. Returns in_maps."""
    import ml_dtypes
    bf = ml_dtypes.bfloat16
    f32 = np.float32
    x = np.asarray(x, f32); y = np.asarray(y, f32)
    r_c = np.asarray(r_c, f32)
    r_q = np.asarray(r_q, f32)
    c_att = np.asarray(c_att_map, f32); q_att = np.asarray(q_att_map, f32)
    W1 = np.asarray(W1, f32); b1 = np.asarray(b1, f32)
    W2 = np.asarray(W2, f32); b2 = np.asarray(b2, f32)
    W3 = np.asarray(W3, f32); b3 = np.asarray(b3, f32)

    # fold b3 into the shipped r tiles: next_r = (r - 0.1*rowsum x b3) - 0.1*att@U'
    if np.any(b3):
        r_c = r_c - 0.1 * c_att.sum(axis=2)[:, None, :, None] * b3
        r_q = r_q - 0.1 * q_att.sum(axis=2)[:, None, :, None] * b3

    # P[b] = [x|y] @ W1[:96] + b1  (k-independent part of layer 1), transposed
    xy = np.concatenate([x, y], axis=-1)                      # [B, C, 96]
    P = xy @ W1[:XD + YD] + b1                                # [B, C, H]
    PT = np.ascontiguousarray(P.transpose(0, 2, 1))           # [B, H, C]
    pt2 = _to_bf16(np.concatenate([PT, PT], axis=2))          # [B, 128, 512]

    # rt[b, g] = [[rcT(2g); rcT(2g+1)] | [rqT(2g); rqT(2g+1)]]  [128, 512]
    rc2 = np.ascontiguousarray(r_c.transpose(0, 1, 3, 2)).reshape(B, C // 2, 128, 256)
    rq2 = np.ascontiguousarray(r_q.transpose(0, 1, 3, 2)).reshape(B, C // 2, 128, 256)
    rt = _to_bf16(np.concatenate([rc2, rq2], axis=3))         # [B, G, 128, 512]

    # attention maps: transposed, chunked along j, pre-scaled by -0.1
    def att_chunks(a):  # [B, i, j] -> [B, 128, 512] = [-0.1*aT ch0 | ch1]
        at = (-0.1 * a.transpose(0, 2, 1)).astype(f32)        # [B, j, i]
        return _to_bf16(np.ascontiguousarray(
            at.reshape(B, 2, 128, 256).transpose(0, 2, 1, 3)).reshape(B, 128, 512))

    acs = att_chunks(c_att)
    aqs = att_chunks(q_att)

    i128 = np.eye(128, dtype=bf)
    w1dA = np.zeros((128, 128), dtype=bf)
    w1dA[:64] = _to_bf16(W1[XD + YD:])
    w1dB = np.zeros((128, 128), dtype=bf)
    w1dB[64:] = _to_bf16(W1[XD + YD:])
    b2_as_bf = np.ascontiguousarray(b2.astype(f32)).view(np.uint16).reshape(128, 2)

    in_maps = []
    for core in range(NCORES):
        b = core // 2
        g0 = (core % 2) * NG_CORE
        cbig = np.zeros((128, CB), dtype=bf)
        cbig[:, 0:512] = pt2[b]
        cbig[:, 512:1024] = acs[b]
        cbig[:, 1024:1536] = aqs[b]
        cbig[:, 1536:1664] = i128
        cbig[:, 1664:1792] = w1dA
        cbig[:, 1792:1920] = w1dB
        cbig[:, 1920:2048] = _to_bf16(W2)
        cbig[:, 2048:2112] = _to_bf16(W3)
        cbig[:, 2112:2114] = b2_as_bf.view(bf)
        # pack 2 consecutive groups side by side on the free dim
        rt_core = rt[b, g0:g0 + NG_CORE].reshape(NIT, 2, 128, 512)
        rt_core = np.ascontiguousarray(
            rt_core.transpose(0, 2, 1, 3)).reshape(NIT, 128, 1024)
        in_maps.append({
            "rt": rt_core,
            "cbig": cbig,
        })
    return in_maps


def _host_post(results):
    """results[core]["out"] [NIT, 128, 1024] bf16 -> (next_r_c, next_r_q)."""
    next_r_c = np.empty((B, C, C, E), np.float32)
    next_r_q = np.empty((B, C, C, E), np.float32)
    for core in range(NCORES):
        out = np.asarray(results[core]["out"]).astype(np.float32)
        out = out.reshape(NIT, 128, 2, 512).transpose(0, 2, 1, 3)
        out = out.reshape(NG_CORE, 128, 512)
        b = core // 2
        k0 = (core % 2) * 128
        rc = out[:, :, 0:256].reshape(NG_CORE, 2, 64, 256)
        rq = out[:, :, 256:512].reshape(NG_CORE, 2, 64, 256)
        next_r_c[b, k0:k0 + 128] = rc.transpose(0, 1, 3, 2).reshape(128, 256, 64)
        next_r_q[b, k0:k0 + 128] = rq.transpose(0, 1, 3, 2).reshape(128, 256, 64)
    return next_r_c, next_r_q


def kernel(x, y, r_c, r_q, c_att_map, q_att_map, W1, b1, W2, b2, W3, b3,
           _trace=False, _trace_kwargs=None):
    import time
    from concourse.bass_utils import run_bass_kernel_spmd

    t0 = time.time()
    nc = _get_nc()
    t1 = time.time()
    in_maps = _host_prep(x, y, r_c, r_q, c_att_map, q_att_map,
                         W1, b1, W2, b2, W3, b3)
    t2 = time.time()
    res = run_bass_kernel_spmd(
        nc, in_maps, list(range(NCORES)),
        trace=_trace, **(_trace_kwargs or {}))
    t3 = time.time()
    out = _host_post(res.results)
    t4 = time.time()
    kernel.last_result = res
    kernel.timings = {"build": t1 - t0, "prep": t2 - t1, "run": t3 - t2,
                      "post": t4 - t3}
    return out
